# revision 5
# baseline (speedup 1.0000x reference)
"""Trainium2 Bass kernel for nn_EnhancedLocalAttention.

Reference semantics (B=4, L=4096, C=1024, H=16, D=64, WIN=256, step=128):
  qkv = x @ W_qkv + b_qkv -> q,k,v [B,H,L,D]
  overlapping windows n: tokens [n*128, n*128+256)
  per (b,h,n): S = (Q_win^T K_win)/8  (D x D, contracted over the 256 window
  tokens), P = softmax(S, axis=-1), O = P @ V_win^T  (D x W)
  regroup: rows of reshape(O, [256, 64]) laid at tokens n*256..n*256+255,
  slice to L -> only windows 0..15 survive; then @ W_out + b_out.

Sharding: 8 cores = (4 batches) x (2 window-halves of 8 windows each).
Each core consumes 9 x 128-token chunks and produces 2048 output rows.

Pipeline: rounds r=0..9 emit QKV for chunk r interleaved at unit granularity
with attention+out-proj for window r-2, so the in-order PE stream always has
dense independent matmul work between the serial softmax chains (keeps the
PE HAM clock-gate warm).
"""

import threading

import numpy as np

import concourse.bacc as bacc
import concourse.masks as masks
import concourse.mybir as mybir
import concourse.tile as tile
from concourse._compat import get_trn_type
from concourse.bass_utils import run_bass_kernel_spmd

F32 = mybir.dt.float32
F32R = mybir.dt.float32r
F16 = mybir.dt.float16
EXP = mybir.ActivationFunctionType.Exp

B, L, C = 4, 4096, 1024
H, D, WIN, STEP = 16, 64, 256, 128
NCHUNK = 9            # 128-token chunks per core
NWIN = 8              # windows per core
TOK = NCHUNK * 128    # 1152 input tokens per core
OUT_ROWS = NWIN * 256 # 2048 output rows per core


def interleave(a, b):
    """Merge two unit lists proportionally (Bresenham)."""
    if not b:
        return list(a)
    if not a:
        return list(b)
    out = []
    ia = ib = 0
    while ia < len(a) or ib < len(b):
        if ib >= len(b) or (ia < len(a) and ia * len(b) <= ib * len(a)):
            out.append(a[ia]); ia += 1
        else:
            out.append(b[ib]); ib += 1
    return out


def build_program():
    nc = bacc.Bacc(
        get_trn_type() or "TRN2",
        target_bir_lowering=False,
        debug=False,
        num_devices=8,
    )
    xs = nc.dram_tensor("xs", [TOK, C], F32, kind="ExternalInput")
    wqkv = nc.dram_tensor("wqkv", [C, 3 * C], F32, kind="ExternalInput")
    bqkv = nc.dram_tensor("bqkv", [3 * C], F32, kind="ExternalInput")
    wout = nc.dram_tensor("wout", [C, C], F32, kind="ExternalInput")
    bout = nc.dram_tensor("bout", [C], F32, kind="ExternalInput")
    out = nc.dram_tensor("out", [OUT_ROWS, C], F32, kind="ExternalOutput")

    from contextlib import ExitStack

    with tile.TileContext(nc) as tc, ExitStack() as ctx:
        pool = lambda name, bufs: ctx.enter_context(tc.tile_pool(name=name, bufs=bufs))
        wq_pool = pool("wq", 8)
        wo_pool = pool("wo", 8)
        const_pool = pool("const", 1)
        x_pool = pool("x", 2)
        xt_pool = pool("xt", 10)
        q_pool = pool("q", 4)
        k_pool = pool("k", 4)
        v_pool = pool("v", 2)
        vt_pool = pool("vt", 28)
        at_pool = pool("at", 4)
        st_pool = pool("st", 6)
        yt_pool = pool("yt", 9)
        o_pool = pool("o", 2)
        ps = ctx.enter_context(tc.tile_pool(name="ps", bufs=8, space="PSUM"))

        # --- constants / weights ---
        idf32 = const_pool.tile([128, 128], F32, tag="idf32", name="idf32")
        masks.make_identity(nc, idf32[:])
        ones_f = const_pool.tile([1, 128], F32, tag="ones_f", name="ones_f")
        nc.vector.memset(ones_f[:], 1.0)
        ones = const_pool.tile([1, 128], F32R, tag="ones", name="ones")
        nc.vector.tensor_copy(ones[:], ones_f[:])
        bq_sb = const_pool.tile([1, 3 * C], F32R, tag="bq", name="bq_sb")
        nc.sync.dma_start(bq_sb[:], bqkv.ap().rearrange("(a f) -> a f", a=1).bitcast(F32R))
        bo_sb = const_pool.tile([1, C], F32R, tag="bo", name="bo_sb")
        nc.sync.dma_start(bo_sb[:], bout.ap().rearrange("(a f) -> a f", a=1).bitcast(F32R))

        wq_sb = []
        for cb in range(8):
            t = wq_pool.tile([128, 3 * C], F32R, tag="wq", name=f"wq{cb}")
            nc.sync.dma_start(t[:], wqkv.ap()[cb * 128 : (cb + 1) * 128, :].bitcast(F32R))
            wq_sb.append(t)
        wo_sb = []
        for cb in range(8):
            t = wo_pool.tile([128, C], F32R, tag="wo", name=f"wo{cb}")
            nc.sync.dma_start(t[:], wout.ap()[cb * 128 : (cb + 1) * 128, :].bitcast(F32R))
            wo_sb.append(t)

        q_sb = [None] * NCHUNK
        k_sb = [None] * NCHUNK
        vt_sb = [[None] * 8 for _ in range(NCHUNK)]

        def qkv_units(r):
            """Emit-callback units for chunk r's QKV projection."""
            st = {}

            def u_load():
                x_t = x_pool.tile([128, C], F32, tag="x", name="x_t")
                nc.sync.dma_start(x_t[:], xs.ap()[r * 128 : (r + 1) * 128, :])
                st["x"] = x_t
                st["xt"] = []
                for cb in range(8):
                    tp = ps.tile([128, 128], F32, tag="ps", name="tp")
                    nc.tensor.transpose(
                        tp[:], x_t[:, cb * 128 : (cb + 1) * 128], idf32[:]
                    )
                    xtt = xt_pool.tile([128, 128], F32R, tag="xt", name="xtt")
                    nc.vector.tensor_copy(xtt[:], tp[:])
                    st["xt"].append(xtt)

            def u_qk_alloc():
                st["pq"] = [
                    ps.tile([128, 512], F32, tag="ps", name=f"pq{i}") for i in range(4)
                ]

            def u_qk(cb):
                def f():
                    for i in range(4):
                        nc.tensor.matmul(
                            st["pq"][i][:],
                            st["xt"][cb][:],
                            wq_sb[cb][:, i * 512 : (i + 1) * 512],
                            start=(cb == 0),
                            stop=False,
                        )
                return f

            def u_qk_fin():
                for i in range(4):
                    nc.tensor.matmul(
                        st["pq"][i][:],
                        ones[:, :],
                        bq_sb[:, i * 512 : (i + 1) * 512],
                        start=False,
                        stop=True,
                    )
                qt = q_pool.tile([128, C], F16, tag="q", name="qt")
                nc.vector.tensor_scalar_mul(qt[:, 0:512], st["pq"][0][:], 0.125)
                nc.vector.tensor_scalar_mul(qt[:, 512:1024], st["pq"][1][:], 0.125)
                q_sb[r] = qt
                kt = k_pool.tile([128, C], F16, tag="k", name="kt")
                nc.vector.tensor_copy(kt[:, 0:512], st["pq"][2][:])
                nc.vector.tensor_copy(kt[:, 512:1024], st["pq"][3][:])
                k_sb[r] = kt

            def u_v_alloc():
                st["pv"] = [
                    ps.tile([128, 512], F32, tag="ps", name=f"pv{i}") for i in range(2)
                ]

            def u_v(cb):
                def f():
                    for i in range(2):
                        nc.tensor.matmul(
                            st["pv"][i][:],
                            st["xt"][cb][:],
                            wq_sb[cb][:, 2048 + i * 512 : 2048 + (i + 1) * 512],
                            start=(cb == 0),
                            stop=False,
                        )
                return f

            def u_v_fin():
                for i in range(2):
                    nc.tensor.matmul(
                        st["pv"][i][:],
                        ones[:, :],
                        bq_sb[:, 2048 + i * 512 : 2048 + (i + 1) * 512],
                        start=False,
                        stop=True,
                    )
                v_t = v_pool.tile([128, C], F16, tag="v", name="v_t")
                nc.vector.tensor_copy(v_t[:, 0:512], st["pv"][0][:])
                nc.vector.tensor_copy(v_t[:, 512:1024], st["pv"][1][:])
                st["v"] = v_t

            def u_vt():
                for fb in range(8):
                    vtt = vt_pool.tile([128, 128], F16, tag="vt", name="vtt")
                    nc.sync.dma_start(
                        vtt[:],
                        st["v"][:, fb * 128 : (fb + 1) * 128],
                        transpose=True,
                    )
                    vt_sb[r][fb] = vtt

            units = [u_load, u_qk_alloc]
            units += [u_qk(cb) for cb in range(8)]
            units += [u_qk_fin, u_v_alloc]
            units += [u_v(cb) for cb in range(8)]
            units += [u_v_fin, u_vt]
            return units

        def window_units(r):
            """Emit-callback units for window r (chunks r, r+1)."""
            yt = []

            def u_hp(hp):
                def f():
                    h0 = 2 * hp
                    s = ps.tile([128, 128], F32, tag="ps", name="s")
                    for rr, (b0, b1) in ((r, (True, False)), (r + 1, (False, True))):
                        nc.tensor.matmul(
                            s[:],
                            q_sb[rr][:, hp * 128 : (hp + 1) * 128],
                            k_sb[rr][:, hp * 128 : (hp + 1) * 128],
                            start=b0,
                            stop=b1,
                        )
                    p_exp = at_pool.tile([128, 64], F16, tag="p_exp", name="p_exp")
                    ssum = st_pool.tile([128, 1], F32, tag="ssum", name="ssum")
                    nc.scalar.activation(
                        p_exp[0:64, :], s[0:64, 0:64], EXP, accum_out=ssum[0:64, :]
                    )
                    nc.scalar.activation(
                        p_exp[64:128, :],
                        s[64:128, 64:128],
                        EXP,
                        accum_out=ssum[64:128, :],
                    )
                    rs = st_pool.tile([128, 1], F32, tag="rs", name="rs")
                    nc.vector.reciprocal(rs[:], ssum[:])
                    pn2 = at_pool.tile([128, 128], F16, tag="pn2", name="pn2")
                    nc.vector.tensor_scalar_mul(
                        pn2[0:64, 0:64], p_exp[0:64, :], rs[0:64, :]
                    )
                    nc.vector.tensor_scalar_mul(
                        pn2[64:128, 64:128], p_exp[64:128, :], rs[64:128, :]
                    )
                    # transpose diag blocks: P^T_h0 -> partitions 0:64,
                    # P^T_h1 -> partitions 64:128 (off-diag is don't-care)
                    ptsb = at_pool.tile([128, 128], F16, tag="ptsb", name="ptsb")
                    nc.sync.dma_start(ptsb[:], pn2[:], transpose=True)

                    ypsum = ps.tile([128, 256], F32, tag="ps", name="ypsum")
                    for h, po in ((h0, 0), (h0 + 1, 64)):
                        rh = ptsb[po : po + 64, po : po + 64]
                        for wq in range(4):
                            vtt = vt_sb[r + wq // 2][h // 2]
                            nc.tensor.matmul(
                                ypsum[po : po + 64, wq * 64 : (wq + 1) * 64],
                                vtt[po : po + 64, (wq % 2) * 64 : (wq % 2) * 64 + 64],
                                rh,
                                start=True,
                                stop=True,
                            )
                    ytt = yt_pool.tile([128, 256], F32R, tag="yt", name="ytt")
                    # Y^T[c, d*4+wq] = ypsum[c, wq*64+d]  (torch-unfold regroup)
                    nc.vector.tensor_copy(
                        ytt[:].rearrange("p (b a) -> p a b", a=4),
                        ypsum[:].rearrange("p (a b) -> p a b", a=4),
                    )
                    yt.append(ytt)
                return f

            def u_op(th):
                def f():
                    po_m = [
                        ps.tile([128, 512], F32, tag="ps", name=f"pom{i}")
                        for i in range(2)
                    ]
                    for cb in range(8):
                        for mi in range(2):
                            nc.tensor.matmul(
                                po_m[mi][:],
                                yt[cb][:, th * 128 : (th + 1) * 128],
                                wo_sb[cb][:, mi * 512 : (mi + 1) * 512],
                                start=(cb == 0),
                                stop=False,
                            )
                    for mi in range(2):
                        nc.tensor.matmul(
                            po_m[mi][:],
                            ones[:, :],
                            bo_sb[:, mi * 512 : (mi + 1) * 512],
                            start=False,
                            stop=True,
                        )
                    ot = o_pool.tile([128, C], F32, tag="o", name="ot")
                    nc.vector.tensor_copy(ot[:, 0:512], po_m[0][:])
                    nc.vector.tensor_copy(ot[:, 512:1024], po_m[1][:])
                    row = r * 256 + th * 128
                    nc.sync.dma_start(out.ap()[row : row + 128, :], ot[:])
                return f

            return [u_hp(hp) for hp in range(8)] + [u_op(0), u_op(1)]

        for r in range(NCHUNK + 1):
            qk = qkv_units(r) if r < NCHUNK else []
            win = window_units(r - 2) if 2 <= r < NWIN + 2 else []
            for u in interleave(qk, win):
                u()

    nc.compile()
    return nc


_CACHE = {}
_LOCK = threading.Lock()


def _get_program():
    with _LOCK:
        if "nc" not in _CACHE:
            _CACHE["nc"] = build_program()
        return _CACHE["nc"]


def kernel(x, W_qkv, b_qkv, W_out, b_out):
    x = np.asarray(x, dtype=np.float32)
    W_qkv = np.asarray(W_qkv, dtype=np.float32)
    b_qkv = np.asarray(b_qkv, dtype=np.float32)
    W_out = np.asarray(W_out, dtype=np.float32)
    b_out = np.asarray(b_out, dtype=np.float32)

    nc = _get_program()
    in_maps = []
    for cid in range(8):
        b, half = cid // 2, cid % 2
        t0 = half * NWIN * STEP
        in_maps.append(
            {
                "xs": np.ascontiguousarray(x[b, t0 : t0 + TOK, :]),
                "wqkv": W_qkv,
                "bqkv": b_qkv,
                "wout": W_out,
                "bout": b_out,
            }
        )
    res = run_bass_kernel_spmd(nc, in_maps, core_ids=list(range(8)))
    out_full = np.empty((B, L, C), dtype=np.float32)
    for cid in range(8):
        b, half = cid // 2, cid % 2
        out_full[b, half * OUT_ROWS : (half + 1) * OUT_ROWS, :] = res.results[cid][
            "out"
        ]
    return out_full


# revision 6
# speedup vs baseline: 1.5360x; 1.5360x over previous
"""Trainium2 Bass kernel for nn_EnhancedLocalAttention.

Reference semantics (B=4, L=4096, C=1024, H=16, D=64, WIN=256, step=128):
  qkv = x @ W_qkv + b_qkv -> q,k,v [B,H,L,D]
  overlapping windows n: tokens [n*128, n*128+256)
  per (b,h,n): S = (Q_win^T K_win)/8  (D x D, contracted over the 256 window
  tokens), P = softmax(S, axis=-1), O = P @ V_win^T  (D x W)
  regroup: rows of reshape(O, [256, 64]) laid at tokens n*256..n*256+255,
  slice to L -> only windows 0..15 survive; then @ W_out + b_out.

Sharding: 8 cores = (4 batches) x (2 window-halves of 8 windows each).
Each core consumes 9 x 128-token chunks and produces 2048 output rows.

Pipeline: rounds r=0..9 emit QKV for chunk r interleaved at unit granularity
with attention+out-proj for window r-2, so the in-order PE stream always has
dense independent matmul work between the serial softmax chains (keeps the
PE HAM clock-gate warm).
"""

import threading

import numpy as np

import concourse.bacc as bacc
import concourse.masks as masks
import concourse.mybir as mybir
import concourse.tile as tile
from concourse._compat import get_trn_type
from concourse.bass_utils import run_bass_kernel_spmd

F32 = mybir.dt.float32
F32R = mybir.dt.float32r
F16 = mybir.dt.float16
EXP = mybir.ActivationFunctionType.Exp

B, L, C = 4, 4096, 1024
H, D, WIN, STEP = 16, 64, 256, 128
NCHUNK = 9            # 128-token chunks per core
NWIN = 8              # windows per core
TOK = NCHUNK * 128    # 1152 input tokens per core
OUT_ROWS = NWIN * 256 # 2048 output rows per core


def interleave(a, b):
    """Merge two unit lists proportionally (Bresenham)."""
    if not b:
        return list(a)
    if not a:
        return list(b)
    out = []
    ia = ib = 0
    while ia < len(a) or ib < len(b):
        if ib >= len(b) or (ia < len(a) and ia * len(b) <= ib * len(a)):
            out.append(a[ia]); ia += 1
        else:
            out.append(b[ib]); ib += 1
    return out


def build_program():
    nc = bacc.Bacc(
        get_trn_type() or "TRN2",
        target_bir_lowering=False,
        debug=False,
        num_devices=8,
    )
    xs = nc.dram_tensor("xs", [TOK, C], F32, kind="ExternalInput")
    wqkv = nc.dram_tensor("wqkv", [C, 3 * C], F32, kind="ExternalInput")
    bqkv = nc.dram_tensor("bqkv", [3 * C], F32, kind="ExternalInput")
    wout = nc.dram_tensor("wout", [C, C], F32, kind="ExternalInput")
    bout = nc.dram_tensor("bout", [C], F32, kind="ExternalInput")
    out = nc.dram_tensor("out", [OUT_ROWS, C], F32, kind="ExternalOutput")

    from contextlib import ExitStack

    with tile.TileContext(nc) as tc, ExitStack() as ctx:
        pool = lambda name, bufs: ctx.enter_context(tc.tile_pool(name=name, bufs=bufs))
        wq_pool = pool("wq", 8)
        wo_pool = pool("wo", 8)
        const_pool = pool("const", 1)
        x_pool = pool("x", 2)
        xt_pool = pool("xt", 10)
        q_pool = pool("q", 4)
        k_pool = pool("k", 4)
        v_pool = pool("v", 2)
        vt_pool = pool("vt", 28)
        at_pool = pool("at", 4)
        st_pool = pool("st", 6)
        yt_pool = pool("yt", 9)
        o_pool = pool("o", 2)
        ps = ctx.enter_context(tc.tile_pool(name="ps", bufs=8, space="PSUM"))

        # --- constants / weights ---
        idf32 = const_pool.tile([128, 128], F32, tag="idf32", name="idf32")
        masks.make_identity(nc, idf32[:])
        idf16 = const_pool.tile([128, 128], F16, tag="idf16", name="idf16")
        masks.make_identity(nc, idf16[:])
        ones_f = const_pool.tile([1, 128], F32, tag="ones_f", name="ones_f")
        nc.vector.memset(ones_f[:], 1.0)
        ones = const_pool.tile([1, 128], F32R, tag="ones", name="ones")
        nc.vector.tensor_copy(ones[:], ones_f[:])
        bq_sb = const_pool.tile([1, 3 * C], F32R, tag="bq", name="bq_sb")
        nc.sync.dma_start(bq_sb[:], bqkv.ap().rearrange("(a f) -> a f", a=1).bitcast(F32R))
        bo_sb = const_pool.tile([1, C], F32R, tag="bo", name="bo_sb")
        nc.sync.dma_start(bo_sb[:], bout.ap().rearrange("(a f) -> a f", a=1).bitcast(F32R))

        wq_sb = []
        for cb in range(8):
            t = wq_pool.tile([128, 3 * C], F32R, tag="wq", name=f"wq{cb}")
            nc.sync.dma_start(t[:], wqkv.ap()[cb * 128 : (cb + 1) * 128, :].bitcast(F32R))
            wq_sb.append(t)
        wo_sb = []
        for cb in range(8):
            t = wo_pool.tile([128, C], F32R, tag="wo", name=f"wo{cb}")
            nc.sync.dma_start(t[:], wout.ap()[cb * 128 : (cb + 1) * 128, :].bitcast(F32R))
            wo_sb.append(t)

        q_sb = [None] * NCHUNK
        k_sb = [None] * NCHUNK
        vt_sb = [[None] * 8 for _ in range(NCHUNK)]

        def qkv_units(r):
            """Emit-callback units for chunk r's QKV projection."""
            st = {}

            def u_load():
                x_t = x_pool.tile([128, C], F32, tag="x", name="x_t")
                nc.sync.dma_start(x_t[:], xs.ap()[r * 128 : (r + 1) * 128, :])
                st["x"] = x_t
                st["xt"] = []
                for cb in range(8):
                    tp = ps.tile([128, 128], F32, tag="ps", name="tp")
                    nc.tensor.transpose(
                        tp[:], x_t[:, cb * 128 : (cb + 1) * 128], idf32[:]
                    )
                    xtt = xt_pool.tile([128, 128], F32R, tag="xt", name="xtt")
                    nc.vector.tensor_copy(xtt[:], tp[:])
                    st["xt"].append(xtt)

            def u_qk_alloc():
                st["pq"] = [
                    ps.tile([128, 512], F32, tag="ps", name=f"pq{i}") for i in range(4)
                ]

            def u_qk(cb):
                def f():
                    for i in range(4):
                        nc.tensor.matmul(
                            st["pq"][i][:],
                            st["xt"][cb][:],
                            wq_sb[cb][:, i * 512 : (i + 1) * 512],
                            start=(cb == 0),
                            stop=False,
                        )
                return f

            def u_qk_fin():
                for i in range(4):
                    nc.tensor.matmul(
                        st["pq"][i][:],
                        ones[:, :],
                        bq_sb[:, i * 512 : (i + 1) * 512],
                        start=False,
                        stop=True,
                    )
                qt = q_pool.tile([128, C], F16, tag="q", name="qt")
                nc.vector.tensor_scalar_mul(qt[:, 0:512], st["pq"][0][:], 0.125)
                nc.vector.tensor_scalar_mul(qt[:, 512:1024], st["pq"][1][:], 0.125)
                q_sb[r] = qt
                kt = k_pool.tile([128, C], F16, tag="k", name="kt")
                nc.vector.tensor_copy(kt[:, 0:512], st["pq"][2][:])
                nc.vector.tensor_copy(kt[:, 512:1024], st["pq"][3][:])
                k_sb[r] = kt

            def u_v_alloc():
                st["pv"] = [
                    ps.tile([128, 512], F32, tag="ps", name=f"pv{i}") for i in range(2)
                ]

            def u_v(cb):
                def f():
                    for i in range(2):
                        nc.tensor.matmul(
                            st["pv"][i][:],
                            st["xt"][cb][:],
                            wq_sb[cb][:, 2048 + i * 512 : 2048 + (i + 1) * 512],
                            start=(cb == 0),
                            stop=False,
                        )
                return f

            def u_v_fin():
                for i in range(2):
                    nc.tensor.matmul(
                        st["pv"][i][:],
                        ones[:, :],
                        bq_sb[:, 2048 + i * 512 : 2048 + (i + 1) * 512],
                        start=False,
                        stop=True,
                    )
                v_t = v_pool.tile([128, C], F16, tag="v", name="v_t")
                nc.vector.tensor_copy(v_t[:, 0:512], st["pv"][0][:])
                nc.vector.tensor_copy(v_t[:, 512:1024], st["pv"][1][:])
                st["v"] = v_t

            def u_vt(fb0):
                def f():
                    for fb in (fb0, fb0 + 1):
                        tpv = ps.tile([128, 128], F16, tag="ps", name="tpv")
                        nc.tensor.transpose(
                            tpv[:], st["v"][:, fb * 128 : (fb + 1) * 128], idf16[:]
                        )
                        vtt = vt_pool.tile([128, 128], F16, tag="vt", name="vtt")
                        nc.vector.tensor_copy(vtt[:], tpv[:])
                        vt_sb[r][fb] = vtt
                return f

            units = [u_load, u_qk_alloc]
            units += [u_qk(cb) for cb in range(8)]
            units += [u_qk_fin, u_v_alloc]
            units += [u_v(cb) for cb in range(8)]
            units += [u_v_fin]
            units += [u_vt(fb0) for fb0 in (0, 2, 4, 6)]
            return units

        def window_units(r):
            """Emit-callback units for window r (chunks r, r+1). Each head
            pair is two units (S+softmax, then P^T+O) so interleaved QKV work
            fills the PE while the softmax chain drains."""
            yt = [None] * 8
            hps = [{} for _ in range(8)]

            def u_hp_s(hp):
                def f():
                    st = hps[hp]
                    s = ps.tile([128, 128], F32, tag="ps", name="s")
                    for rr, (b0, b1) in ((r, (True, False)), (r + 1, (False, True))):
                        nc.tensor.matmul(
                            s[:],
                            q_sb[rr][:, hp * 128 : (hp + 1) * 128],
                            k_sb[rr][:, hp * 128 : (hp + 1) * 128],
                            start=b0,
                            stop=b1,
                        )
                    p_exp = at_pool.tile([128, 64], F16, tag="p_exp", name="p_exp")
                    ssum = st_pool.tile([128, 1], F32, tag="ssum", name="ssum")
                    nc.scalar.activation(
                        p_exp[0:64, :], s[0:64, 0:64], EXP, accum_out=ssum[0:64, :]
                    )
                    nc.scalar.activation(
                        p_exp[64:128, :],
                        s[64:128, 64:128],
                        EXP,
                        accum_out=ssum[64:128, :],
                    )
                    rs = st_pool.tile([128, 1], F32, tag="rs", name="rs")
                    nc.vector.reciprocal(rs[:], ssum[:])
                    p_n = at_pool.tile([128, 64], F16, tag="p_n", name="p_n")
                    nc.vector.tensor_scalar_mul(p_n[:], p_exp[:], rs[:])
                    st["p_n"] = p_n
                return f

            def u_hp_o(hp):
                def f():
                    st = hps[hp]
                    h0 = 2 * hp
                    p_n = st["p_n"]
                    ptp = ps.tile([128, 64], F16, tag="ps", name="ptp")
                    nc.tensor.transpose(
                        ptp[0:64, :], p_n[0:64, :], idf16[0:64, 0:64]
                    )
                    nc.tensor.transpose(
                        ptp[64:128, :], p_n[64:128, :], idf16[64:128, 64:128]
                    )
                    ptsb = at_pool.tile([128, 64], F16, tag="ptsb", name="ptsb")
                    nc.vector.tensor_copy(ptsb[:], ptp[:])

                    ypsum = ps.tile([128, 256], F32, tag="ps", name="ypsum")
                    for h, po in ((h0, 0), (h0 + 1, 64)):
                        rh = ptsb[po : po + 64, :]
                        for wq in range(4):
                            vtt = vt_sb[r + wq // 2][h // 2]
                            nc.tensor.matmul(
                                ypsum[po : po + 64, wq * 64 : (wq + 1) * 64],
                                vtt[po : po + 64, (wq % 2) * 64 : (wq % 2) * 64 + 64],
                                rh,
                                start=True,
                                stop=True,
                            )
                    ytt = yt_pool.tile([128, 256], F32R, tag="yt", name="ytt")
                    # Y^T[c, d*4+wq] = ypsum[c, wq*64+d]  (torch-unfold regroup)
                    nc.vector.tensor_copy(
                        ytt[:].rearrange("p (b a) -> p a b", a=4),
                        ypsum[:].rearrange("p (a b) -> p a b", a=4),
                    )
                    yt[hp] = ytt
                return f

            def u_op(th):
                def f():
                    po_m = [
                        ps.tile([128, 512], F32, tag="ps", name=f"pom{i}")
                        for i in range(2)
                    ]
                    for cb in range(8):
                        for mi in range(2):
                            nc.tensor.matmul(
                                po_m[mi][:],
                                yt[cb][:, th * 128 : (th + 1) * 128],
                                wo_sb[cb][:, mi * 512 : (mi + 1) * 512],
                                start=(cb == 0),
                                stop=False,
                            )
                    for mi in range(2):
                        nc.tensor.matmul(
                            po_m[mi][:],
                            ones[:, :],
                            bo_sb[:, mi * 512 : (mi + 1) * 512],
                            start=False,
                            stop=True,
                        )
                    ot = o_pool.tile([128, C], F32, tag="o", name="ot")
                    nc.vector.tensor_copy(ot[:, 0:512], po_m[0][:])
                    nc.vector.tensor_copy(ot[:, 512:1024], po_m[1][:])
                    row = r * 256 + th * 128
                    nc.sync.dma_start(out.ap()[row : row + 128, :], ot[:])
                return f

            units = [u_hp_s(0)]
            for hp in range(1, 8):
                units += [u_hp_s(hp), u_hp_o(hp - 1)]
            units += [u_hp_o(7), u_op(0), u_op(1)]
            return units

        for r in range(NCHUNK + 1):
            qk = qkv_units(r) if r < NCHUNK else []
            win = window_units(r - 2) if 2 <= r < NWIN + 2 else []
            for u in interleave(qk, win):
                u()

    nc.compile()
    return nc


_CACHE = {}
_LOCK = threading.Lock()


def _get_program():
    with _LOCK:
        if "nc" not in _CACHE:
            _CACHE["nc"] = build_program()
        return _CACHE["nc"]


def kernel(x, W_qkv, b_qkv, W_out, b_out):
    x = np.asarray(x, dtype=np.float32)
    W_qkv = np.asarray(W_qkv, dtype=np.float32)
    b_qkv = np.asarray(b_qkv, dtype=np.float32)
    W_out = np.asarray(W_out, dtype=np.float32)
    b_out = np.asarray(b_out, dtype=np.float32)

    nc = _get_program()
    in_maps = []
    for cid in range(8):
        b, half = cid // 2, cid % 2
        t0 = half * NWIN * STEP
        in_maps.append(
            {
                "xs": np.ascontiguousarray(x[b, t0 : t0 + TOK, :]),
                "wqkv": W_qkv,
                "bqkv": b_qkv,
                "wout": W_out,
                "bout": b_out,
            }
        )
    res = run_bass_kernel_spmd(nc, in_maps, core_ids=list(range(8)))
    out_full = np.empty((B, L, C), dtype=np.float32)
    for cid in range(8):
        b, half = cid // 2, cid % 2
        out_full[b, half * OUT_ROWS : (half + 1) * OUT_ROWS, :] = res.results[cid][
            "out"
        ]
    return out_full


# revision 9
# speedup vs baseline: 1.7555x; 1.1429x over previous
"""Trainium2 Bass kernel for nn_EnhancedLocalAttention.

Reference semantics (B=4, L=4096, C=1024, H=16, D=64, WIN=256, step=128):
  qkv = x @ W_qkv + b_qkv -> q,k,v [B,H,L,D]
  overlapping windows n: tokens [n*128, n*128+256)
  per (b,h,n): S = (Q_win^T K_win)/8  (D x D, contracted over the 256 window
  tokens), P = softmax(S, axis=-1), O = P @ V_win^T  (D x W)
  regroup: rows of reshape(O, [256, 64]) laid at tokens n*256..n*256+255,
  slice to L -> only windows 0..15 survive; then @ W_out + b_out.

Sharding: 8 cores = (4 batches) x (2 window-halves of 8 windows each).
Each core consumes 9 x 128-token chunks and produces 2048 output rows.

Pipeline: rounds r=0..9 emit QKV for chunk r interleaved at unit granularity
with attention+out-proj for window r-2, so the in-order PE stream always has
dense independent matmul work between the serial softmax chains (keeps the
PE HAM clock-gate warm).
"""

import threading

import numpy as np

import concourse.bacc as bacc
import concourse.masks as masks
import concourse.mybir as mybir
import concourse.tile as tile
from concourse._compat import get_trn_type
from concourse.bass_utils import run_bass_kernel_spmd

F32 = mybir.dt.float32
F32R = mybir.dt.float32r
F16 = mybir.dt.float16
EXP = mybir.ActivationFunctionType.Exp

B, L, C = 4, 4096, 1024
H, D, WIN, STEP = 16, 64, 256, 128
NCHUNK = 9            # 128-token chunks per core
NWIN = 8              # windows per core
TOK = NCHUNK * 128    # 1152 input tokens per core
OUT_ROWS = NWIN * 256 # 2048 output rows per core


def interleave(a, b):
    """Merge two unit lists proportionally (Bresenham)."""
    if not b:
        return list(a)
    if not a:
        return list(b)
    out = []
    ia = ib = 0
    while ia < len(a) or ib < len(b):
        if ib >= len(b) or (ia < len(a) and ia * len(b) <= ib * len(a)):
            out.append(a[ia]); ia += 1
        else:
            out.append(b[ib]); ib += 1
    return out


def build_program(with_bias=True):
    nc = bacc.Bacc(
        get_trn_type() or "TRN2",
        target_bir_lowering=False,
        debug=False,
        num_devices=8,
    )
    xs = nc.dram_tensor("xs", [TOK, C], F32, kind="ExternalInput")
    wqkv = nc.dram_tensor("wqkv", [C, 3 * C], F32, kind="ExternalInput")
    bqkv = nc.dram_tensor("bqkv", [3 * C], F32, kind="ExternalInput")
    wout = nc.dram_tensor("wout", [C, C], F32, kind="ExternalInput")
    bout = nc.dram_tensor("bout", [C], F32, kind="ExternalInput")
    out = nc.dram_tensor("out", [OUT_ROWS, C], F32, kind="ExternalOutput")

    from contextlib import ExitStack

    with tile.TileContext(nc) as tc, ExitStack() as ctx:
        pool = lambda name, bufs: ctx.enter_context(tc.tile_pool(name=name, bufs=bufs))
        wq_pool = pool("wq", 8)
        wo_pool = pool("wo", 8)
        const_pool = pool("const", 1)
        x_pool = pool("x", 3)
        xt_pool = pool("xt", 8)
        q_pool = pool("q", 4)
        k_pool = pool("k", 4)
        v_pool = pool("v", 2)
        vt_pool = pool("vt", 28)
        at_pool = pool("at", 3)
        st_pool = pool("st", 6)
        yt_pool = pool("yt", 9)
        o_pool = pool("o", 2)
        ps = ctx.enter_context(tc.tile_pool(name="ps", bufs=8, space="PSUM"))

        # --- constants / weights ---
        idf32 = const_pool.tile([128, 128], F32, tag="idf32", name="idf32")
        masks.make_identity(nc, idf32[:])
        idf16 = const_pool.tile([128, 128], F16, tag="idf16", name="idf16")
        masks.make_identity(nc, idf16[:])
        ones_f = const_pool.tile([1, 128], F32, tag="ones_f", name="ones_f")
        nc.vector.memset(ones_f[:], 1.0)
        ones = const_pool.tile([1, 128], F32R, tag="ones", name="ones")
        nc.vector.tensor_copy(ones[:], ones_f[:])
        bq_sb = const_pool.tile([1, 3 * C], F32R, tag="bq", name="bq_sb")
        nc.sync.dma_start(bq_sb[:], bqkv.ap().rearrange("(a f) -> a f", a=1).bitcast(F32R))
        bo_sb = const_pool.tile([1, C], F32R, tag="bo", name="bo_sb")
        nc.sync.dma_start(bo_sb[:], bout.ap().rearrange("(a f) -> a f", a=1).bitcast(F32R))

        x_pre = [None] * NCHUNK

        def prefetch_x(r):
            x_t = x_pool.tile([128, C], F32, tag="x", name="x_t")
            nc.sync.dma_start(x_t[:], xs.ap()[r * 128 : (r + 1) * 128, :])
            x_pre[r] = x_t

        prefetch_x(0)
        wq_sb = []
        for cb in range(8):
            t = wq_pool.tile([128, 3 * C], F32R, tag="wq", name=f"wq{cb}")
            nc.sync.dma_start(t[:], wqkv.ap()[cb * 128 : (cb + 1) * 128, :].bitcast(F32R))
            wq_sb.append(t)
        wo_sb = []
        for cb in range(8):
            t = wo_pool.tile([128, C], F32R, tag="wo", name=f"wo{cb}")
            nc.sync.dma_start(t[:], wout.ap()[cb * 128 : (cb + 1) * 128, :].bitcast(F32R))
            wo_sb.append(t)

        q_sb = [None] * NCHUNK
        k_sb = [None] * NCHUNK
        vt_sb = [[None] * 8 for _ in range(NCHUNK)]

        def qkv_units(r):
            """Emit-callback units for chunk r's QKV projection."""
            st = {}

            def u_load():
                if r + 1 < NCHUNK:
                    prefetch_x(r + 1)
                x_t = x_pre[r]
                st["x"] = x_t
                st["xt"] = []
                for cb in range(8):
                    tp = ps.tile([128, 128], F32, tag="ps", name="tp")
                    nc.tensor.transpose(
                        tp[:], x_t[:, cb * 128 : (cb + 1) * 128], idf32[:]
                    )
                    xtt = xt_pool.tile([128, 128], F32R, tag="xt", name="xtt")
                    nc.vector.tensor_copy(xtt[:], tp[:])
                    st["xt"].append(xtt)

            def u_qk_alloc():
                st["pq"] = [
                    ps.tile([128, 512], F32, tag="ps", name=f"pq{i}") for i in range(4)
                ]

            def u_qk(cb):
                def f():
                    for i in range(4):
                        nc.tensor.matmul(
                            st["pq"][i][:],
                            st["xt"][cb][:],
                            wq_sb[cb][:, i * 512 : (i + 1) * 512],
                            start=(cb == 0),
                            stop=(not with_bias and cb == 7),
                        )
                return f

            def u_qk_fin():
                if with_bias:
                    for i in range(4):
                        nc.tensor.matmul(
                            st["pq"][i][:],
                            ones[:, :],
                            bq_sb[:, i * 512 : (i + 1) * 512],
                            start=False,
                            stop=True,
                        )
                qt = q_pool.tile([128, C], F16, tag="q", name="qt")
                nc.vector.tensor_scalar_mul(qt[:, 0:512], st["pq"][0][:], 0.125)
                nc.vector.tensor_scalar_mul(qt[:, 512:1024], st["pq"][1][:], 0.125)
                q_sb[r] = qt
                kt = k_pool.tile([128, C], F16, tag="k", name="kt")
                nc.vector.tensor_copy(kt[:, 0:512], st["pq"][2][:])
                nc.vector.tensor_copy(kt[:, 512:1024], st["pq"][3][:])
                k_sb[r] = kt

            def u_v_alloc():
                st["pv"] = [
                    ps.tile([128, 512], F32, tag="ps", name=f"pv{i}") for i in range(2)
                ]

            def u_v(cb):
                def f():
                    for i in range(2):
                        nc.tensor.matmul(
                            st["pv"][i][:],
                            st["xt"][cb][:],
                            wq_sb[cb][:, 2048 + i * 512 : 2048 + (i + 1) * 512],
                            start=(cb == 0),
                            stop=(not with_bias and cb == 7),
                        )
                return f

            def u_v_fin():
                if with_bias:
                    for i in range(2):
                        nc.tensor.matmul(
                            st["pv"][i][:],
                            ones[:, :],
                            bq_sb[:, 2048 + i * 512 : 2048 + (i + 1) * 512],
                            start=False,
                            stop=True,
                        )
                v_t = v_pool.tile([128, C], F16, tag="v", name="v_t")
                nc.vector.tensor_copy(v_t[:, 0:512], st["pv"][0][:])
                nc.vector.tensor_copy(v_t[:, 512:1024], st["pv"][1][:])
                st["v"] = v_t

            def u_vt(fb0):
                def f():
                    for fb in (fb0, fb0 + 1):
                        tpv = ps.tile([128, 128], F16, tag="ps", name="tpv")
                        nc.tensor.transpose(
                            tpv[:], st["v"][:, fb * 128 : (fb + 1) * 128], idf16[:]
                        )
                        vtt = vt_pool.tile([128, 128], F16, tag="vt", name="vtt")
                        nc.vector.tensor_copy(vtt[:], tpv[:])
                        vt_sb[r][fb] = vtt
                return f

            units = [u_load, u_qk_alloc]
            units += [u_qk(cb) for cb in range(8)]
            units += [u_qk_fin, u_v_alloc]
            units += [u_v(cb) for cb in range(8)]
            units += [u_v_fin]
            units += [u_vt(fb0) for fb0 in (0, 2, 4, 6)]
            return units

        def window_units(r):
            """Emit-callback units for window r (chunks r, r+1). Each head
            pair is two units (S+softmax, then P^T+O) so interleaved QKV work
            fills the PE while the softmax chain drains."""
            yt = [None] * 8
            hps = [{} for _ in range(8)]

            def u_hp_s(hp):
                def f():
                    st = hps[hp]
                    s = ps.tile([128, 128], F32, tag="ps", name="s")
                    for rr, (b0, b1) in ((r, (True, False)), (r + 1, (False, True))):
                        nc.tensor.matmul(
                            s[:],
                            q_sb[rr][:, hp * 128 : (hp + 1) * 128],
                            k_sb[rr][:, hp * 128 : (hp + 1) * 128],
                            start=b0,
                            stop=b1,
                        )
                    p_exp = at_pool.tile([128, 64], F16, tag="p_exp", name="p_exp")
                    ssum = st_pool.tile([128, 1], F32, tag="ssum", name="ssum")
                    nc.scalar.activation(
                        p_exp[0:64, :], s[0:64, 0:64], EXP, accum_out=ssum[0:64, :]
                    )
                    nc.scalar.activation(
                        p_exp[64:128, :],
                        s[64:128, 64:128],
                        EXP,
                        accum_out=ssum[64:128, :],
                    )
                    rs = st_pool.tile([128, 1], F32, tag="rs", name="rs")
                    nc.vector.reciprocal(rs[:], ssum[:])
                    p_n = at_pool.tile([128, 64], F16, tag="p_n", name="p_n")
                    nc.vector.tensor_scalar_mul(p_n[:], p_exp[:], rs[:])
                    st["p_n"] = p_n
                return f

            def u_hp_o(hp):
                def f():
                    st = hps[hp]
                    h0 = 2 * hp
                    p_n = st["p_n"]
                    ptp = ps.tile([128, 64], F16, tag="ps", name="ptp")
                    nc.tensor.transpose(
                        ptp[0:64, :], p_n[0:64, :], idf16[0:64, 0:64]
                    )
                    nc.tensor.transpose(
                        ptp[64:128, :], p_n[64:128, :], idf16[64:128, 64:128]
                    )
                    ptsb = at_pool.tile([128, 64], F16, tag="ptsb", name="ptsb")
                    nc.vector.tensor_copy(ptsb[:], ptp[:])

                    ypsum = ps.tile([128, 256], F32, tag="ps", name="ypsum")
                    for h, po in ((h0, 0), (h0 + 1, 64)):
                        rh = ptsb[po : po + 64, :]
                        for wq in range(4):
                            vtt = vt_sb[r + wq // 2][h // 2]
                            nc.tensor.matmul(
                                ypsum[po : po + 64, wq * 64 : (wq + 1) * 64],
                                vtt[po : po + 64, (wq % 2) * 64 : (wq % 2) * 64 + 64],
                                rh,
                                start=True,
                                stop=True,
                            )
                    ytt = yt_pool.tile([128, 256], F32R, tag="yt", name="ytt")
                    # Y^T[c, d*4+wq] = ypsum[c, wq*64+d]  (torch-unfold regroup)
                    nc.vector.tensor_copy(
                        ytt[:].rearrange("p (b a) -> p a b", a=4),
                        ypsum[:].rearrange("p (a b) -> p a b", a=4),
                    )
                    yt[hp] = ytt
                return f

            def u_op(th):
                def f():
                    po_m = [
                        ps.tile([128, 512], F32, tag="ps", name=f"pom{i}")
                        for i in range(2)
                    ]
                    for cb in range(8):
                        for mi in range(2):
                            nc.tensor.matmul(
                                po_m[mi][:],
                                yt[cb][:, th * 128 : (th + 1) * 128],
                                wo_sb[cb][:, mi * 512 : (mi + 1) * 512],
                                start=(cb == 0),
                                stop=(not with_bias and cb == 7),
                            )
                    if with_bias:
                        for mi in range(2):
                            nc.tensor.matmul(
                                po_m[mi][:],
                                ones[:, :],
                                bo_sb[:, mi * 512 : (mi + 1) * 512],
                                start=False,
                                stop=True,
                            )
                    ot = o_pool.tile([128, C], F32, tag="o", name="ot")
                    nc.vector.tensor_copy(ot[:, 0:512], po_m[0][:])
                    nc.vector.tensor_copy(ot[:, 512:1024], po_m[1][:])
                    row = r * 256 + th * 128
                    nc.sync.dma_start(out.ap()[row : row + 128, :], ot[:])
                return f

            units = [u_hp_s(0)]
            for hp in range(1, 8):
                units += [u_hp_s(hp), u_hp_o(hp - 1)]
            units += [u_hp_o(7), u_op(0), u_op(1)]
            return units

        for r in range(NCHUNK + 1):
            qk = qkv_units(r) if r < NCHUNK else []
            win = window_units(r - 2) if 2 <= r < NWIN + 2 else []
            for u in interleave(qk, win):
                u()

    nc.compile()
    return nc


_CACHE = {}
_LOCK = threading.Lock()


def _get_program(with_bias=True):
    key = f"nc_bias{with_bias}"
    with _LOCK:
        if key not in _CACHE:
            _CACHE[key] = build_program(with_bias=with_bias)
        return _CACHE[key]


def kernel(x, W_qkv, b_qkv, W_out, b_out):
    x = np.asarray(x, dtype=np.float32)
    W_qkv = np.asarray(W_qkv, dtype=np.float32)
    b_qkv = np.asarray(b_qkv, dtype=np.float32)
    W_out = np.asarray(W_out, dtype=np.float32)
    b_out = np.asarray(b_out, dtype=np.float32)

    with_bias = bool(np.any(b_qkv)) or bool(np.any(b_out))
    nc = _get_program(with_bias=with_bias)
    in_maps = []
    for cid in range(8):
        b, half = cid // 2, cid % 2
        t0 = half * NWIN * STEP
        in_maps.append(
            {
                "xs": np.ascontiguousarray(x[b, t0 : t0 + TOK, :]),
                "wqkv": W_qkv,
                "bqkv": b_qkv,
                "wout": W_out,
                "bout": b_out,
            }
        )
    res = run_bass_kernel_spmd(nc, in_maps, core_ids=list(range(8)))
    out_full = np.empty((B, L, C), dtype=np.float32)
    for cid in range(8):
        b, half = cid // 2, cid % 2
        out_full[b, half * OUT_ROWS : (half + 1) * OUT_ROWS, :] = res.results[cid][
            "out"
        ]
    return out_full


# revision 10
# speedup vs baseline: 1.7556x; 1.0001x over previous
"""Trainium2 Bass kernel for nn_EnhancedLocalAttention.

Reference semantics (B=4, L=4096, C=1024, H=16, D=64, WIN=256, step=128):
  qkv = x @ W_qkv + b_qkv -> q,k,v [B,H,L,D]
  overlapping windows n: tokens [n*128, n*128+256)
  per (b,h,n): S = (Q_win^T K_win)/8  (D x D, contracted over the 256 window
  tokens), P = softmax(S, axis=-1), O = P @ V_win^T  (D x W)
  regroup: rows of reshape(O, [256, 64]) laid at tokens n*256..n*256+255,
  slice to L -> only windows 0..15 survive; then @ W_out + b_out.

Sharding: 8 cores = (4 batches) x (2 window-halves of 8 windows each).
Each core consumes 9 x 128-token chunks and produces 2048 output rows.

Pipeline: rounds r=0..9 emit QKV for chunk r interleaved at unit granularity
with attention+out-proj for window r-2, so the in-order PE stream always has
dense independent matmul work between the serial softmax chains (keeps the
PE HAM clock-gate warm).
"""

import threading

import numpy as np

import concourse.bacc as bacc
import concourse.masks as masks
import concourse.mybir as mybir
import concourse.tile as tile
from concourse._compat import get_trn_type
from concourse.bass_utils import run_bass_kernel_spmd

F32 = mybir.dt.float32
F32R = mybir.dt.float32r
F16 = mybir.dt.float16
EXP = mybir.ActivationFunctionType.Exp

B, L, C = 4, 4096, 1024
H, D, WIN, STEP = 16, 64, 256, 128
NCHUNK = 9            # 128-token chunks per core
NWIN = 8              # windows per core
TOK = NCHUNK * 128    # 1152 input tokens per core
OUT_ROWS = NWIN * 256 # 2048 output rows per core


def interleave(a, b):
    """Merge two unit lists proportionally (Bresenham)."""
    if not b:
        return list(a)
    if not a:
        return list(b)
    out = []
    ia = ib = 0
    while ia < len(a) or ib < len(b):
        if ib >= len(b) or (ia < len(a) and ia * len(b) <= ib * len(a)):
            out.append(a[ia]); ia += 1
        else:
            out.append(b[ib]); ib += 1
    return out


def build_program(with_bias=True):
    nc = bacc.Bacc(
        get_trn_type() or "TRN2",
        target_bir_lowering=False,
        debug=False,
        num_devices=8,
    )
    xs = nc.dram_tensor("xs", [TOK, C], F32, kind="ExternalInput")
    wqkv = nc.dram_tensor("wqkv", [C, 3 * C], F32, kind="ExternalInput")
    bqkv = nc.dram_tensor("bqkv", [3 * C], F32, kind="ExternalInput")
    wout = nc.dram_tensor("wout", [C, C], F32, kind="ExternalInput")
    bout = nc.dram_tensor("bout", [C], F32, kind="ExternalInput")
    out = nc.dram_tensor("out", [OUT_ROWS, C], F32, kind="ExternalOutput")

    from contextlib import ExitStack

    with tile.TileContext(nc) as tc, ExitStack() as ctx:
        pool = lambda name, bufs: ctx.enter_context(tc.tile_pool(name=name, bufs=bufs))
        wq_pool = pool("wq", 8)
        wo_pool = pool("wo", 8)
        const_pool = pool("const", 1)
        x_pool = pool("x", 2)
        xt_pool = pool("xt", 16)
        q_pool = pool("q", 4)
        k_pool = pool("k", 4)
        v_pool = pool("v", 2)
        vt_pool = pool("vt", 28)
        at_pool = pool("at", 3)
        st_pool = pool("st", 6)
        yt_pool = pool("yt", 9)
        o_pool = pool("o", 2)
        ps = ctx.enter_context(tc.tile_pool(name="ps", bufs=8, space="PSUM"))

        # --- constants / weights ---
        idf32 = const_pool.tile([128, 128], F32, tag="idf32", name="idf32")
        masks.make_identity(nc, idf32[:])
        idf16 = const_pool.tile([128, 128], F16, tag="idf16", name="idf16")
        masks.make_identity(nc, idf16[:])
        ones_f = const_pool.tile([1, 128], F32, tag="ones_f", name="ones_f")
        nc.vector.memset(ones_f[:], 1.0)
        ones = const_pool.tile([1, 128], F32R, tag="ones", name="ones")
        nc.vector.tensor_copy(ones[:], ones_f[:])
        bq_sb = const_pool.tile([1, 3 * C], F32R, tag="bq", name="bq_sb")
        nc.sync.dma_start(bq_sb[:], bqkv.ap().rearrange("(a f) -> a f", a=1).bitcast(F32R))
        bo_sb = const_pool.tile([1, C], F32R, tag="bo", name="bo_sb")
        nc.sync.dma_start(bo_sb[:], bout.ap().rearrange("(a f) -> a f", a=1).bitcast(F32R))

        x_pre = [None] * NCHUNK
        xt_all = [None] * NCHUNK

        def prefetch_x(r):
            x_t = x_pool.tile([128, C], F32, tag="x", name="x_t")
            nc.sync.dma_start(x_t[:], xs.ap()[r * 128 : (r + 1) * 128, :])
            x_pre[r] = x_t

        def transpose_x(r):
            xt_all[r] = []
            for cb in range(8):
                tp = ps.tile([128, 128], F32, tag="ps", name="tp")
                nc.tensor.transpose(
                    tp[:], x_pre[r][:, cb * 128 : (cb + 1) * 128], idf32[:]
                )
                xtt = xt_pool.tile([128, 128], F32R, tag="xt", name="xtt")
                nc.vector.tensor_copy(xtt[:], tp[:])
                xt_all[r].append(xtt)

        prefetch_x(0)
        wq_sb = []
        for cb in range(8):
            t = wq_pool.tile([128, 3 * C], F32R, tag="wq", name=f"wq{cb}")
            nc.sync.dma_start(t[:], wqkv.ap()[cb * 128 : (cb + 1) * 128, :].bitcast(F32R))
            wq_sb.append(t)
        wo_sb = []
        for cb in range(8):
            t = wo_pool.tile([128, C], F32R, tag="wo", name=f"wo{cb}")
            nc.sync.dma_start(t[:], wout.ap()[cb * 128 : (cb + 1) * 128, :].bitcast(F32R))
            wo_sb.append(t)

        q_sb = [None] * NCHUNK
        k_sb = [None] * NCHUNK
        vt_sb = [[None] * 8 for _ in range(NCHUNK)]

        def qkv_units(r):
            """Emit-callback units for chunk r's QKV projection."""
            st = {}

            def u_load():
                if r + 1 < NCHUNK:
                    prefetch_x(r + 1)
                if xt_all[r] is None:
                    transpose_x(r)
                st["xt"] = xt_all[r]

            def u_xtail():
                if r + 1 < NCHUNK:
                    transpose_x(r + 1)

            def u_qk_alloc():
                st["pq"] = [
                    ps.tile([128, 512], F32, tag="ps", name=f"pq{i}") for i in range(4)
                ]

            def u_qk(cb):
                def f():
                    for i in range(4):
                        nc.tensor.matmul(
                            st["pq"][i][:],
                            st["xt"][cb][:],
                            wq_sb[cb][:, i * 512 : (i + 1) * 512],
                            start=(cb == 0),
                            stop=(not with_bias and cb == 7),
                        )
                return f

            def u_qk_fin():
                if with_bias:
                    for i in range(4):
                        nc.tensor.matmul(
                            st["pq"][i][:],
                            ones[:, :],
                            bq_sb[:, i * 512 : (i + 1) * 512],
                            start=False,
                            stop=True,
                        )
                qt = q_pool.tile([128, C], F16, tag="q", name="qt")
                nc.vector.tensor_scalar_mul(qt[:, 0:512], st["pq"][0][:], 0.125)
                nc.vector.tensor_scalar_mul(qt[:, 512:1024], st["pq"][1][:], 0.125)
                q_sb[r] = qt
                kt = k_pool.tile([128, C], F16, tag="k", name="kt")
                nc.vector.tensor_copy(kt[:, 0:512], st["pq"][2][:])
                nc.vector.tensor_copy(kt[:, 512:1024], st["pq"][3][:])
                k_sb[r] = kt

            def u_v_alloc():
                st["pv"] = [
                    ps.tile([128, 512], F32, tag="ps", name=f"pv{i}") for i in range(2)
                ]

            def u_v(cb):
                def f():
                    for i in range(2):
                        nc.tensor.matmul(
                            st["pv"][i][:],
                            st["xt"][cb][:],
                            wq_sb[cb][:, 2048 + i * 512 : 2048 + (i + 1) * 512],
                            start=(cb == 0),
                            stop=(not with_bias and cb == 7),
                        )
                return f

            def u_v_fin():
                if with_bias:
                    for i in range(2):
                        nc.tensor.matmul(
                            st["pv"][i][:],
                            ones[:, :],
                            bq_sb[:, 2048 + i * 512 : 2048 + (i + 1) * 512],
                            start=False,
                            stop=True,
                        )
                v_t = v_pool.tile([128, C], F16, tag="v", name="v_t")
                nc.vector.tensor_copy(v_t[:, 0:512], st["pv"][0][:])
                nc.vector.tensor_copy(v_t[:, 512:1024], st["pv"][1][:])
                st["v"] = v_t

            def u_vt(fb0):
                def f():
                    for fb in (fb0, fb0 + 1):
                        tpv = ps.tile([128, 128], F16, tag="ps", name="tpv")
                        nc.tensor.transpose(
                            tpv[:], st["v"][:, fb * 128 : (fb + 1) * 128], idf16[:]
                        )
                        vtt = vt_pool.tile([128, 128], F16, tag="vt", name="vtt")
                        nc.vector.tensor_copy(vtt[:], tpv[:])
                        vt_sb[r][fb] = vtt
                return f

            units = [u_load, u_qk_alloc]
            units += [u_qk(cb) for cb in range(8)]
            units += [u_qk_fin, u_v_alloc]
            units += [u_v(cb) for cb in range(8)]
            units += [u_v_fin]
            units += [u_vt(fb0) for fb0 in (0, 2, 4, 6)]
            units += [u_xtail]
            return units

        def window_units(r):
            """Emit-callback units for window r (chunks r, r+1). Each head
            pair is two units (S+softmax, then P^T+O) so interleaved QKV work
            fills the PE while the softmax chain drains."""
            yt = [None] * 8
            hps = [{} for _ in range(8)]

            def u_hp_s(hp):
                def f():
                    st = hps[hp]
                    s = ps.tile([128, 128], F32, tag="ps", name="s")
                    for rr, (b0, b1) in ((r, (True, False)), (r + 1, (False, True))):
                        nc.tensor.matmul(
                            s[:],
                            q_sb[rr][:, hp * 128 : (hp + 1) * 128],
                            k_sb[rr][:, hp * 128 : (hp + 1) * 128],
                            start=b0,
                            stop=b1,
                        )
                    p_exp = at_pool.tile([128, 64], F16, tag="p_exp", name="p_exp")
                    ssum = st_pool.tile([128, 1], F32, tag="ssum", name="ssum")
                    nc.scalar.activation(
                        p_exp[0:64, :], s[0:64, 0:64], EXP, accum_out=ssum[0:64, :]
                    )
                    nc.scalar.activation(
                        p_exp[64:128, :],
                        s[64:128, 64:128],
                        EXP,
                        accum_out=ssum[64:128, :],
                    )
                    rs = st_pool.tile([128, 1], F32, tag="rs", name="rs")
                    nc.vector.reciprocal(rs[:], ssum[:])
                    p_n = at_pool.tile([128, 64], F16, tag="p_n", name="p_n")
                    nc.vector.tensor_scalar_mul(p_n[:], p_exp[:], rs[:])
                    st["p_n"] = p_n
                return f

            def u_hp_o(hp):
                def f():
                    st = hps[hp]
                    h0 = 2 * hp
                    p_n = st["p_n"]
                    ptp = ps.tile([128, 64], F16, tag="ps", name="ptp")
                    nc.tensor.transpose(
                        ptp[0:64, :], p_n[0:64, :], idf16[0:64, 0:64]
                    )
                    nc.tensor.transpose(
                        ptp[64:128, :], p_n[64:128, :], idf16[64:128, 64:128]
                    )
                    ptsb = at_pool.tile([128, 64], F16, tag="ptsb", name="ptsb")
                    nc.vector.tensor_copy(ptsb[:], ptp[:])

                    ypsum = ps.tile([128, 256], F32, tag="ps", name="ypsum")
                    for h, po in ((h0, 0), (h0 + 1, 64)):
                        rh = ptsb[po : po + 64, :]
                        for wq in range(4):
                            vtt = vt_sb[r + wq // 2][h // 2]
                            nc.tensor.matmul(
                                ypsum[po : po + 64, wq * 64 : (wq + 1) * 64],
                                vtt[po : po + 64, (wq % 2) * 64 : (wq % 2) * 64 + 64],
                                rh,
                                start=True,
                                stop=True,
                            )
                    ytt = yt_pool.tile([128, 256], F32R, tag="yt", name="ytt")
                    # Y^T[c, d*4+wq] = ypsum[c, wq*64+d]  (torch-unfold regroup)
                    nc.vector.tensor_copy(
                        ytt[:].rearrange("p (b a) -> p a b", a=4),
                        ypsum[:].rearrange("p (a b) -> p a b", a=4),
                    )
                    yt[hp] = ytt
                return f

            def u_op(th):
                def f():
                    po_m = [
                        ps.tile([128, 512], F32, tag="ps", name=f"pom{i}")
                        for i in range(2)
                    ]
                    for cb in range(8):
                        for mi in range(2):
                            nc.tensor.matmul(
                                po_m[mi][:],
                                yt[cb][:, th * 128 : (th + 1) * 128],
                                wo_sb[cb][:, mi * 512 : (mi + 1) * 512],
                                start=(cb == 0),
                                stop=(not with_bias and cb == 7),
                            )
                    if with_bias:
                        for mi in range(2):
                            nc.tensor.matmul(
                                po_m[mi][:],
                                ones[:, :],
                                bo_sb[:, mi * 512 : (mi + 1) * 512],
                                start=False,
                                stop=True,
                            )
                    ot = o_pool.tile([128, C], F32, tag="o", name="ot")
                    nc.vector.tensor_copy(ot[:, 0:512], po_m[0][:])
                    nc.vector.tensor_copy(ot[:, 512:1024], po_m[1][:])
                    row = r * 256 + th * 128
                    nc.sync.dma_start(out.ap()[row : row + 128, :], ot[:])
                return f

            units = [u_hp_s(0)]
            for hp in range(1, 8):
                units += [u_hp_s(hp), u_hp_o(hp - 1)]
            units += [u_hp_o(7), u_op(0), u_op(1)]
            return units

        for r in range(NCHUNK + 1):
            qk = qkv_units(r) if r < NCHUNK else []
            win = window_units(r - 2) if 2 <= r < NWIN + 2 else []
            for u in interleave(qk, win):
                u()

    nc.compile()
    return nc


_CACHE = {}
_LOCK = threading.Lock()


def _get_program(with_bias=True):
    key = f"nc_bias{with_bias}"
    with _LOCK:
        if key not in _CACHE:
            _CACHE[key] = build_program(with_bias=with_bias)
        return _CACHE[key]


def kernel(x, W_qkv, b_qkv, W_out, b_out):
    x = np.asarray(x, dtype=np.float32)
    W_qkv = np.asarray(W_qkv, dtype=np.float32)
    b_qkv = np.asarray(b_qkv, dtype=np.float32)
    W_out = np.asarray(W_out, dtype=np.float32)
    b_out = np.asarray(b_out, dtype=np.float32)

    with_bias = bool(np.any(b_qkv)) or bool(np.any(b_out))
    nc = _get_program(with_bias=with_bias)
    in_maps = []
    for cid in range(8):
        b, half = cid // 2, cid % 2
        t0 = half * NWIN * STEP
        in_maps.append(
            {
                "xs": np.ascontiguousarray(x[b, t0 : t0 + TOK, :]),
                "wqkv": W_qkv,
                "bqkv": b_qkv,
                "wout": W_out,
                "bout": b_out,
            }
        )
    res = run_bass_kernel_spmd(nc, in_maps, core_ids=list(range(8)))
    out_full = np.empty((B, L, C), dtype=np.float32)
    for cid in range(8):
        b, half = cid // 2, cid % 2
        out_full[b, half * OUT_ROWS : (half + 1) * OUT_ROWS, :] = res.results[cid][
            "out"
        ]
    return out_full
